# revision 3
# baseline (speedup 1.0000x reference)
"""GNN message-passing kernel v2 for TRN2, 8-core SPMD (self-contained).

Key design vs v1:
- Node tables live in SBUF (bf16, [128, 65 slots x 1KB]; slot 64 = zero row
  for -1 padding). adj/dep gathers are SBUF-source transpose-mode dma_gather
  (feature-major output), avoiding the HBM small-descriptor penalty.
- Edge table stays in HBM (32MB); eid gathers are HBM transpose-mode.
- Neighbor sums run on the PE: identity-stationary matmuls accumulate
  feature-major gather chunks into PSUM (sum of chunks = chunked sum).
  eid -1 padding is remapped to edge row 0 and cancelled by rank-1
  correction matmuls; adj/dep padding uses the zero slot.
- 1/cnt scaling: rank-1 matmul broadcasts the per-node rc row into PSUM;
  one DVE multiply fuses scale + fp32->bf16 cast into the xT staging tile.
- Linear layers are all-bf16 matmuls (xT chunks stationary, W moving).
- Tables are stored block-permuted so chunked AllGathers write contiguous
  DRAM ranges; chunked SBUF reloads follow each AG chunk. Gather indices
  are precomputed host-side against the permuted layout.
"""
import sys

sys.path.insert(0, '/opt/trn_rl_repo')

import numpy as np
import concourse.bass as bass
import concourse.mybir as mybir
from concourse import tile
from concourse.bacc import Bacc
from concourse.masks import make_identity

F32 = mybir.dt.float32
BF16 = mybir.dt.bfloat16
I16 = mybir.dt.int16
P = 128

N, E, D = 8192, 32768, 512
DEG, DEP, K, CORES = 16, 8, 3, 8
NS, ES = N // CORES, E // CORES          # 1024, 4096
NB, EB = NS // P, ES // P                # 8, 32
DC = D // P                              # 4
KCN, KCE = (2 * D) // P, (3 * D) // P    # 8, 12
CSN, CSE = 8, 32                         # AG chunk sizes (blocks): monolithic per table
NJ = 4                                   # idxs per gather call = NJ*128 (<=512: transpose-mode HW limit)
NCN = DEG // NJ                          # adj/eid calls per node block
NCE = DEP // NJ                          # dep calls per dir per edge block
CI = NJ * 8                              # idx cols per call
NQ = NS // 64                            # node sb table: rows per partition


def build(Q_A=1, Q_B=1):
    nc = Bacc("TRN2", target_bir_lowering=False, debug=False, num_devices=CORES,
              num_swdge_queues=4)

    # ---- external inputs (host-permuted / packed; see prep_inputs) ----
    fw_tab0 = nc.dram_tensor("fw_tab0", [N, D], BF16, kind="ExternalInput")
    bw_tab0 = nc.dram_tensor("bw_tab0", [N, D], BF16, kind="ExternalInput")
    e_tab0 = nc.dram_tensor("e_tab0", [E, D], BF16, kind="ExternalInput")
    fw_ownT0 = nc.dram_tensor("fw_ownT0", [NS, D], BF16, kind="ExternalInput")
    bw_ownT0 = nc.dram_tensor("bw_ownT0", [NS, D], BF16, kind="ExternalInput")
    e_ownT0 = nc.dram_tensor("e_ownT0", [ES, D], BF16, kind="ExternalInput")
    adjf_i = nc.dram_tensor("adjf_i", [P, NB * 2 * 64], I16, kind="ExternalInput")
    adjb_i = nc.dram_tensor("adjb_i", [P, NB * 2 * 64], I16, kind="ExternalInput")
    eidf_i = nc.dram_tensor("eidf_i", [P, NB * 2 * 64], I16, kind="ExternalInput")
    eidb_i = nc.dram_tensor("eidb_i", [P, NB * 2 * 64], I16, kind="ExternalInput")
    depf_i = nc.dram_tensor("depf_i", [P, EB * 64], I16, kind="ExternalInput")
    depb_i = nc.dram_tensor("depb_i", [P, EB * 64], I16, kind="ExternalInput")
    rcn4_fw = nc.dram_tensor("rcn4_fw", [NB, D], BF16, kind="ExternalInput")
    rcn4_bw = nc.dram_tensor("rcn4_bw", [NB, D], BF16, kind="ExternalInput")
    rce4_fw = nc.dram_tensor("rce4_fw", [EB, D], BF16, kind="ExternalInput")
    rce4_bw = nc.dram_tensor("rce4_bw", [EB, D], BF16, kind="ExternalInput")
    ce_fw = nc.dram_tensor("ce_fw", [NB, P], BF16, kind="ExternalInput")
    ce_bw = nc.dram_tensor("ce_bw", [NB, P], BF16, kind="ExternalInput")
    wfc = nc.dram_tensor("wfc", [P, KCN * D], BF16, kind="ExternalInput")
    wbc = nc.dram_tensor("wbc", [P, KCN * D], BF16, kind="ExternalInput")
    wedge = nc.dram_tensor("wedge", [P, KCE * D], BF16, kind="ExternalInput")
    bfc = nc.dram_tensor("bfc", [1, D], BF16, kind="ExternalInput")
    bbc = nc.dram_tensor("bbc", [1, D], BF16, kind="ExternalInput")
    bedge = nc.dram_tensor("bedge", [1, D], BF16, kind="ExternalInput")
    fw_out = nc.dram_tensor("fw_out", [NS, D], F32, kind="ExternalOutput")
    bw_out = nc.dram_tensor("bw_out", [NS, D], F32, kind="ExternalOutput")

    rg = [list(range(CORES))]
    MUL = mybir.AluOpType.mult
    RELU = mybir.ActivationFunctionType.Relu
    COPY = mybir.ActivationFunctionType.Copy

    with tile.TileContext(nc) as tc:
        with (
            tc.tile_pool(name="const", bufs=1) as cp,
            tc.tile_pool(name="xp", bufs=2) as xp,
            tc.tile_pool(name="rcsb", bufs=2) as rcsb,
            tc.tile_pool(name="fhp", bufs=2) as fhp,
            tc.tile_pool(name="fp32p", bufs=1) as fp32p,
            tc.tile_pool(name="r0p", bufs=2) as r0p,
            tc.tile_pool(name="ptm", bufs=2, space="PSUM") as ptm,
            tc.tile_pool(name="ptx", bufs=2, space="PSUM") as ptx,
            tc.tile_pool(name="pop", bufs=2, space="PSUM") as pop,
            tc.tile_pool(name="dram", bufs=1, space="DRAM") as dp,
        ):
            # ---- constants ----
            ident = cp.tile([P, P], BF16, name="ident")
            make_identity(nc, ident[:])
            ones1 = cp.tile([1, P], BF16, name="ones1")
            nc.gpsimd.memset(ones1[:], 1.0)

            # SBUF node tables, slot 64 zeroed for -1 padding
            fw_sb = cp.tile([P, 65 * D], BF16, name="fw_sb")
            bw_sb = cp.tile([P, 65 * D], BF16, name="bw_sb")
            nc.gpsimd.memset(fw_sb[:, 64 * D:], 0.0)
            nc.gpsimd.memset(bw_sb[:, 64 * D:], 0.0)
            nc.sync.dma_start(
                out=fw_sb[:, 0:64 * D],
                in_=fw_tab0[:].rearrange("(p s) d -> p (s d)", p=P))
            nc.sync.dma_start(
                out=bw_sb[:, 0:64 * D],
                in_=bw_tab0[:].rearrange("(p s) d -> p (s d)", p=P))

            w_slot = cp.tile([P, KCE * D], BF16, name="w_slot")
            w_node = w_slot[:, 0:KCN * D]
            nc.sync.dma_start(out=w_node, in_=wfc[:])

            def load_c(name, src, shape):
                t = cp.tile(shape, BF16, name=name)
                nc.sync.dma_start(out=t[:], in_=src[:])
                return t

            bfc_t = load_c("bfc_t", bfc, [1, D])
            bbc_t = load_c("bbc_t", bbc, [1, D])
            be_t = load_c("be_t", bedge, [1, D])

            # ---- DRAM internals ----
            fw_tabD = dp.tile([N, D], BF16, name="fw_tabD")
            bw_tabD = dp.tile([N, D], BF16, name="bw_tabD")
            e_tabD = dp.tile([E, D], BF16, name="e_tabD")
            fw_shB = [dp.tile([NS, D], BF16, name=f"fw_shB{i}") for i in range(2)]
            bw_shB = [dp.tile([NS, D], BF16, name=f"bw_shB{i}") for i in range(2)]
            e_shB = [dp.tile([ES, D], BF16, name=f"e_shB{i}") for i in range(2)]

            def load_r0e(src):
                t = r0p.tile([1, D], BF16, name="r0e", tag="r0e")
                nc.sync.dma_start(out=t[:], in_=src[0:1, :])
                return t

            r0e = load_r0e(e_tab0)

            def idx_load(pool, src, col0):
                t = pool.tile([P, CI], I16, name="ix", tag="ix")
                nc.sync.dma_start(out=t[:], in_=src[:, col0:col0 + CI])
                return t

            def gather_sb(pool, q, tab_sb, ixt):
                g = pool.tile([P, DC * NJ * P], BF16, name="g", tag="g")
                nc.gpsimd.dma_gather(
                    out_ap=g[:].rearrange("p (c i) -> p c i", i=NJ * P),
                    in_ap=tab_sb[:],
                    idxs_ap=ixt[:], num_idxs=NJ * P, num_idxs_reg=NJ * P,
                    elem_size=D, queue_num=q, transpose=True,
                    sbuf_tokens_per_rank=128, sbuf_free_dim_per_rank=2 * D,
                )
                return g

            def gather_hbm(pool, q, tab, ixt):
                g = pool.tile([P, DC * NJ * P], BF16, name="g", tag="g")
                nc.gpsimd.dma_gather(
                    out_ap=g[:].rearrange("p (c i) -> p c i", i=NJ * P),
                    in_ap=tab[:],
                    idxs_ap=ixt[:], num_idxs=NJ * P, num_idxs_reg=NJ * P,
                    elem_size=D, queue_num=q, transpose=True,
                )
                return g

            def row_load(pool, tag, src, b, w):
                t = pool.tile([1, w], BF16, name=tag, tag=tag)
                nc.sync.dma_start(out=t[:], in_=src[b:b + 1, :])
                return t

            def accum_mean(psum, gts, first, last_corr=None):
                """psum[:, c*128:(c+1)*128] += sum_j gt[:, c, j*128:(j+1)*128]
                (+ rank-1 correction) for each feature chunk c."""
                for c in range(DC):
                    base = c * (NJ * P)
                    for ti, gt in enumerate(gts):
                        for j in range(NJ):
                            nc.tensor.matmul(
                                out=psum[:, c * P:(c + 1) * P], lhsT=ident[:],
                                rhs=gt[:, base + j * P:base + (j + 1) * P],
                                start=(ti == 0 and j == 0),
                                stop=(last_corr is None and ti == len(gts) - 1
                                      and j == NJ - 1),
                            )
                    if last_corr is not None:
                        r0row, crow = last_corr
                        nc.tensor.matmul(
                            out=psum[:, c * P:(c + 1) * P],
                            lhsT=r0row[:, c * P:(c + 1) * P], rhs=crow,
                            start=False, stop=True,
                        )

            def scale_to(xt, coloff, psum_m, rc_row):
                ps_rc = ptx.tile([P, D], F32, name="ps_rc", tag="ps_rc")
                nc.tensor.matmul(out=ps_rc[:], lhsT=ones1[:], rhs=rc_row,
                                 start=True, stop=True)
                rc_sb = rcsb.tile([P, D], BF16, name="rc_sb", tag="rc_sb")
                nc.scalar.activation(out=rc_sb[:], in_=ps_rc[:], func=COPY)
                nc.vector.tensor_tensor(
                    out=xt[:, coloff:coloff + D], in0=psum_m[:], in1=rc_sb[:],
                    op=MUL)

            def own_transposed(xt, src_dram, b, ownT0):
                if ownT0 is not None:
                    nc.sync.dma_start(out=xt[:, 0:D],
                                      in_=ownT0[b * P:(b + 1) * P, :])
                    return
                fprev = fhp.tile([P, D], BF16, name="fprev", tag="fprev")
                nc.sync.dma_start(out=fprev[:],
                                  in_=src_dram[b * P:(b + 1) * P, :])
                ps_t = ptx.tile([P, D], BF16, name="ps_t", tag="ps_t")
                for c in range(DC):
                    nc.tensor.transpose(
                        out=ps_t[:, c * P:(c + 1) * P],
                        in_=fprev[:, c * P:(c + 1) * P], identity=ident[:])
                nc.scalar.activation(out=xt[:, 0:D], in_=ps_t[:], func=COPY)

            def linear(xt, kc, w_t, b_row, relu, out_sb, out_f32=None):
                w_t = [w_slot[:, kk * D:(kk + 1) * D] for kk in range(kc)]
                ps = pop.tile([P, D], F32, name="ps_o", tag="ps_o")
                for kk in range(kc):
                    nc.tensor.matmul(
                        out=ps[:], lhsT=xt[:, kk * P:(kk + 1) * P],
                        rhs=w_t[kk],
                        start=(kk == 0), stop=False)
                nc.tensor.matmul(out=ps[:], lhsT=ones1[:], rhs=b_row[:],
                                 start=False, stop=True)
                fn = RELU if relu else COPY
                if out_sb is not None:
                    nc.scalar.activation(out=out_sb[:], in_=ps[:], func=fn)
                if out_f32 is not None:
                    nc.scalar.activation(out=out_f32[:], in_=ps[:], func=fn)

            def node_block(k, b, gpT, gpE, ipT, ipE, rcp,
                           tab_sb, etab, adj_src, eid_src, ownT0, own_dram,
                           rc_src, ce_src, b_row, dst_sh, dst32):
                # SBUF adj gathers (Q_A) + HBM eid gathers (Q_B); the global
                # SWDGE queue pattern must stay [A,A,B,B] (sem lanes are
                # assigned round-robin in program order and lock to a queue).
                gts = []
                for h in range(NCN):
                    ixt = idx_load(ipT, adj_src, (b * NCN + h) * CI)
                    gts.append(gather_sb(gpT, Q_A, tab_sb, ixt))
                for h in range(NCN):
                    ixt = idx_load(ipE, eid_src, (b * NCN + h) * CI)
                    gts.append(gather_hbm(gpE, Q_B, etab, ixt))

                xt = xp.tile([P, KCE * P], BF16, name="xt", tag="xt")
                own_transposed(xt, own_dram, b, ownT0)

                ce_row = row_load(rcp, "ce", ce_src, b, P)
                rc_row = row_load(rcp, "rc", rc_src, b, D)
                ps_m = ptm.tile([P, D], F32, name="ps_m", tag="ps_m")
                accum_mean(ps_m, gts, True,
                           last_corr=(r0e, ce_row[:]))
                scale_to(xt, D, ps_m, rc_row[:])

                fh = None
                if dst_sh is not None:
                    fh = fhp.tile([P, D], BF16, name="fh", tag="fh")
                f32t = None
                if dst32 is not None:
                    f32t = fp32p.tile([P, D], F32, name="f32t", tag="f32t")
                linear(xt, KCN, None, b_row, dst32 is None, fh, f32t)
                if fh is not None:
                    nc.sync.dma_start(out=dst_sh[b * P:(b + 1) * P, :], in_=fh[:])
                if f32t is not None:
                    nc.sync.dma_start(out=dst32[b * P:(b + 1) * P, :], in_=f32t[:])

            def edge_block(k, b, gpF, gpB, ipF, ipB, rcp, ownT0, own_dram,
                           dst_sh):
                gfs = []
                for h in range(NCE):
                    ixf = idx_load(ipF, depf_i, (b * NCE + h) * CI)
                    gfs.append(gather_sb(gpF, Q_A, fw_sb, ixf))
                gbs = []
                for h in range(NCE):
                    ixb = idx_load(ipB, depb_i, (b * NCE + h) * CI)
                    gbs.append(gather_sb(gpB, Q_B, bw_sb, ixb))
                xt = xp.tile([P, KCE * P], BF16, name="xte", tag="xt")
                own_transposed(xt, own_dram, b, ownT0)
                rcf_row = row_load(rcp, "rc", rce4_fw, b, D)
                ps_f = ptm.tile([P, D], F32, name="ps_f", tag="ps_m")
                accum_mean(ps_f, gfs, True)
                scale_to(xt, D, ps_f, rcf_row[:])
                rcb_row = row_load(rcp, "rc", rce4_bw, b, D)
                ps_b = ptm.tile([P, D], F32, name="ps_b", tag="ps_m")
                accum_mean(ps_b, gbs, True)
                scale_to(xt, 2 * D, ps_b, rcb_row[:])
                es = fhp.tile([P, D], BF16, name="es", tag="fh")
                linear(xt, KCE, None, be_t, True, es)
                nc.sync.dma_start(out=dst_sh[b * P:(b + 1) * P, :], in_=es[:])

            def allgather(src, dst):
                nc.gpsimd.collective_compute(
                    "AllGather", mybir.AluOpType.bypass, replica_groups=rg,
                    ins=[src], outs=[dst])

            # ================= main schedule =================
            for k in range(K):
                last = (k == K - 1)
                sh_w = k % 2          # staging written this hop
                sh_r = (k + 1) % 2    # staging read for own rows (k>0)
                etab_k = e_tab0 if k == 0 else e_tabD

                for (dirname, tab_sb, adj_src, eid_src, ownT0, own_dram,
                     rc_src, ce_src, w_src, b_row, shB, tabD, out32) in (
                    ("fw", fw_sb, adjf_i, eidf_i,
                     fw_ownT0 if k == 0 else None,
                     None if k == 0 else fw_shB[sh_r],
                     rcn4_fw, ce_fw, wfc, bfc_t, fw_shB, fw_tabD, fw_out),
                    ("bw", bw_sb, adjb_i, eidb_i,
                     bw_ownT0 if k == 0 else None,
                     None if k == 0 else bw_shB[sh_r],
                     rcn4_bw, ce_bw, wbc, bbc_t, bw_shB, bw_tabD, bw_out),
                ):
                    if dirname == "bw" or k > 0:
                        nc.sync.dma_start(out=w_node, in_=w_src[:])
                    with (
                        tc.tile_pool(name="gpT", bufs=5) as gpT,
                        tc.tile_pool(name="gpE", bufs=5) as gpE,
                        tc.tile_pool(name="ipT", bufs=4) as ipT,
                        tc.tile_pool(name="ipE", bufs=4) as ipE,
                        tc.tile_pool(name="rcp", bufs=3) as rcp,
                    ):
                        for b in range(NB):
                            node_block(k, b, gpT, gpE, ipT, ipE, rcp,
                                       tab_sb, etab_k, adj_src, eid_src,
                                       ownT0, own_dram, rc_src, ce_src, b_row,
                                       None if last else shB[sh_w],
                                       out32 if last else None)
                            if not last and (b + 1) % CSN == 0:
                                q = b // CSN
                                allgather(
                                    shB[sh_w][q * CSN * P:(q + 1) * CSN * P, :],
                                    tabD[q * CSN * P * CORES:
                                         (q + 1) * CSN * P * CORES, :])
                        # table reloads AFTER all this hop's gathers from
                        # tab_sb (they bring in epoch k+1 for the edge phase
                        # and next hop; issuing earlier would corrupt later
                        # blocks' epoch-k gathers)
                        if not last:
                            for q in range(NB // CSN):
                                nc.sync.dma_start(
                                    out=tab_sb[q * CSN * P // 64 * CORES:
                                               (q + 1) * CSN * P // 64 * CORES,
                                               0:64 * D],
                                    in_=tabD[q * CSN * P * CORES:
                                             (q + 1) * CSN * P * CORES, :]
                                    .rearrange("(p s) d -> p (s d)", s=64))

                if not last:
                    nc.sync.dma_start(out=w_slot[:], in_=wedge[:])
                    with (
                        tc.tile_pool(name="gpF", bufs=4) as gpF,
                        tc.tile_pool(name="gpB", bufs=4) as gpB,
                        tc.tile_pool(name="ipF", bufs=4) as ipF,
                        tc.tile_pool(name="ipB", bufs=4) as ipB,
                        tc.tile_pool(name="rcpE", bufs=3) as rcpE,
                    ):
                        for b in range(EB):
                            edge_block(k, b, gpF, gpB, ipF, ipB, rcpE,
                                       e_ownT0 if k == 0 else None,
                                       None if k == 0 else e_shB[sh_r],
                                       e_shB[sh_w])
                            if (b + 1) % CSE == 0:
                                q = b // CSE
                                allgather(
                                    e_shB[sh_w][q * CSE * P:(q + 1) * CSE * P, :],
                                    e_tabD[q * CSE * P * CORES:
                                           (q + 1) * CSE * P * CORES, :])
                    r0e = load_r0e(e_tabD)

    nc.compile()
    return nc
